# revision 45
# baseline (speedup 1.0000x reference)
"""Trainium2 Bass kernel for an attention-LSTM decoder (scan over 128 steps).

Data-parallel over batch: 64 batches -> 8 cores x 8 batches. All weights and
the per-core encoder slice live SBUF-resident in bf16; the 128-step recurrence
is fully unrolled with feature-major (transposed) activation layouts so every
matmul has its contraction dim on partitions.

The kernel is PE *instruction-dispatch* bound: with an 8-wide batch free dim,
every LDWEIGHTS+MATMUL pair costs ~31-37 ns regardless of weight columns,
dtype, or tile packing (measured via mmbench.py: fp8 weights, 4x col-tiling,
M=32 stationaries, and weight reuse all land within ~1 ns of bf16 base), so
total time ~= (matmuls per step) x (dispatch floor) and the design minimizes
matmul count and keeps the PE queue fed:
- W_out is folded into the attention weight (scores depend on the previous
  out = Wou@h1 + bo only linearly): scores = h0@Wa_h0' + h1@(Wa_out Wou)',
  K drops 1536->1024 and the output path leaves the recurrence entirely.
- softmax normalization is deferred: the context matmuls consume raw masked
  exps; the per-batch 1/sum lands once on the context tile (partition-
  replicated row sums via one all-ones-stationary matmul).
- gate biases are rank-16 indicator matmuls that OPEN each gate PSUM bank's
  accumulation group (they zero the bank), so the group close waits only on
  the last Wi matmul. (DVE pre-writes cannot replace them: only TensorE
  sets PSUM has_written bits, a later matmul would overwrite.)
- the Wh0 @ h0 matmuls for step t+1 and the (lagged) W_out matmuls for step
  t-1 are emitted where the PE would otherwise wait on the cell chains.
- the step output is stored feature-major straight from SBUF (no PE
  transpose); the host does the final untranspose + f32 upcast.

Self-contained: hardcodes all shapes; imports the Bass/Tile stack from the
machine-wide /opt/trn_rl_repo checkout.
"""
import sys

sys.path.insert(0, "/opt/trn_rl_repo")
import contextlib

import ml_dtypes
import numpy as np

import concourse.bacc as bacc
import concourse.bass as bass
import concourse.tile as tile
from concourse import mybir

import hashlib


def _bust_dim(*args):
    """PJRT's NEFF cache fingerprints the HLO without the custom call's
    backend_config (where the Bass BIR lives), so different kernels with the
    same I/O signature alias to one cached NEFF. Encode a hash of this file +
    the build args into an (otherwise unused) input tensor's shape so every
    kernel revision gets a distinct cache key."""
    h = hashlib.sha256()
    try:
        with open(__file__, "rb") as f:
            h.update(f.read())
    except OSError:
        pass
    h.update(repr(args).encode())
    return int.from_bytes(h.digest()[:4], "little") % 251 + 1


B, ENC, DEC, H = 64, 256, 128, 512
IN = 2 * H
NCORES = 8
BL = B // NCORES  # 8 batches per core

F32 = mybir.dt.float32
BF16 = mybir.dt.bfloat16
FP8 = mybir.dt.float8e4
TANH = mybir.ActivationFunctionType.Tanh
EXP = mybir.ActivationFunctionType.Exp
MULT = mybir.AluOpType.mult
ADD = mybir.AluOpType.add


def _split_cols(ap, a, b):
    """View a [P, a*b] AP as [P, a, b] (row-major split of the free dim)."""
    ap2 = ap.copy()
    ap2.ap = ap.ap[0:1] + [[b, a], [1, b]]
    return ap2


def _bcast_cols(ap, a, b):
    """View a [P, b] AP as [P, a, b] with the middle dim broadcast."""
    return ap.unsqueeze(1).broadcast_to((ap.shape[0], a, b))


def build_nc(dec=DEC, unroll=2, loop_mult=1, enc8=False, pipe=2, abl="",
             hints=0):
    # pipe: 0 = Wh0 at the top of each step; 1 = Wh0 for step k+1 emitted in
    # step k's tail only within the unrolled body (no state crosses the For_i
    # back-edge); 2 = fully pipelined across iterations (prologue + epilogue).
    # abl: debugging ablations — "mulloop" (per-dt normalization muls instead
    # of the broadcast-AP multiply), "f32y" (f32 transpose + output path).
    EDT = FP8 if enc8 else BF16
    nc = bacc.Bacc("TRN2", num_devices=NCORES, debug=False)

    d_wi0 = nc.dram_tensor("wi0", [128, 8 * 4 * H], BF16, kind="ExternalInput")
    d_wh0 = nc.dram_tensor("wh0", [128, 4 * 4 * H], BF16, kind="ExternalInput")
    d_wi1 = nc.dram_tensor("wi1", [128, 4 * 4 * H], BF16, kind="ExternalInput")
    d_wh1 = nc.dram_tensor("wh1", [128, 4 * 4 * H], BF16, kind="ExternalInput")
    d_wat = nc.dram_tensor("wat", [128, 8 * ENC], BF16, kind="ExternalInput")
    d_wou = nc.dram_tensor("wou", [128, 4 * IN], BF16, kind="ExternalInput")
    d_enc = nc.dram_tensor("enc", [128, BL * 2 * IN], EDT, kind="ExternalInput")
    d_msk = nc.dram_tensor("msk", [128, 16], F32, kind="ExternalInput")
    d_bat = nc.dram_tensor("bat", [128, 4], F32, kind="ExternalInput")
    d_b0k = nc.dram_tensor("b0k", [16, 128], BF16, kind="ExternalInput")
    d_b1k = nc.dram_tensor("b1k", [16, 128], BF16, kind="ExternalInput")
    d_ind = nc.dram_tensor("ind", [16, 128], BF16, kind="ExternalInput")
    d_bo = nc.dram_tensor("bo", [128, 64], F32, kind="ExternalInput")
    d_id = nc.dram_tensor("id128", [128, 128], BF16, kind="ExternalInput")
    d_b0f = nc.dram_tensor("b0f", [128, 128], F32, kind="ExternalInput")
    d_b1f = nc.dram_tensor("b1f", [128, 128], F32, kind="ExternalInput")
    d_idf = nc.dram_tensor("idf", [128, 128], F32, kind="ExternalInput")
    d_mneg = nc.dram_tensor("mneg", [16, 128], BF16, kind="ExternalInput")
    d_eye16 = nc.dram_tensor("eye16", [16, 16], BF16, kind="ExternalInput")
    nc.dram_tensor(
        "bust",
        [1, _bust_dim(dec, unroll, loop_mult, enc8, pipe, abl, hints)],
        F32,
        kind="ExternalInput",
    )
    # y layout: [p, t*64 + g*8 + b] with output element (b, t, 128*g + p),
    # bf16, feature-major; the host does the final untranspose.
    d_y = nc.dram_tensor("y", [128, dec * 64], BF16, kind="ExternalOutput")

    with tile.TileContext(nc) as tc:
        with contextlib.ExitStack() as ctx:
            cpool = ctx.enter_context(tc.tile_pool(name="cpool", bufs=1))
            state = ctx.enter_context(tc.tile_pool(name="state", bufs=1))
            work = ctx.enter_context(tc.tile_pool(name="work", bufs=3))
            psum = ctx.enter_context(tc.tile_pool(name="psum", bufs=1, space="PSUM"))

            # ---- load constants ----
            def load(dram, shape, dtype, nsplit=1, tag=None):
                t = cpool.tile(shape, dtype, tag=tag)
                cols = shape[1]
                step = cols // nsplit
                for i in range(nsplit):
                    nc.gpsimd.dma_start(
                        t[:, i * step : (i + 1) * step],
                        dram[:, i * step : (i + 1) * step],
                    )
                return t

            wi0 = load(d_wi0, [128, 8 * 4 * H], BF16, nsplit=4, tag="wi0")
            wh0 = load(d_wh0, [128, 4 * 4 * H], BF16, nsplit=2, tag="wh0")
            wi1 = load(d_wi1, [128, 4 * 4 * H], BF16, nsplit=2, tag="wi1")
            wh1 = load(d_wh1, [128, 4 * 4 * H], BF16, nsplit=2, tag="wh1")
            wat = load(d_wat, [128, 8 * ENC], BF16, tag="wat")
            wou = load(d_wou, [128, 4 * IN], BF16, tag="wou")
            enc = load(d_enc, [128, BL * 2 * IN], EDT, nsplit=4, tag="enc")
            msk = load(d_msk, [128, 16], F32, tag="msk")
            bat = load(d_bat, [128, 4], F32, tag="bat")
            b0k = load(d_b0k, [16, 128], BF16, tag="b0k")
            b1k = load(d_b1k, [16, 128], BF16, tag="b1k")
            ind = load(d_ind, [16, 128], BF16, tag="ind")
            bo = load(d_bo, [128, 64], F32, tag="bo")
            id128 = load(d_id, [128, 128], BF16, tag="id128")
            b0f = load(d_b0f, [128, 128], F32, tag="b0f")
            b1f = load(d_b1f, [128, 128], F32, tag="b1f")
            idf = load(d_idf, [128, 128], F32, tag="idf")
            mneg = load(d_mneg, [16, 128], BF16, tag="mneg")
            eye16 = load(d_eye16, [16, 16], BF16, tag="eye16")
            ones128 = cpool.tile([128, 128], BF16)
            nc.vector.memset(ones128, 1.0)
            ones128f = cpool.tile([128, 128], F32)
            nc.vector.memset(ones128f, 1.0)
            rec128 = cpool.tile([128, 8], F32, tag="rec128")

            # ---- recurrent state (feature-major) ----
            c0 = state.tile([128, 32], F32)
            c1 = state.tile([128, 32], F32)
            h0 = state.tile([128, 32], BF16)
            h1 = state.tile([128, 32], BF16)
            for t in (c0, c1, h0, h1):
                nc.vector.memset(t, 0.0)

            # ---- psum banks: ONE tile instance each, reused every step, so
            # accumulation groups that span steps (the pipelined Wh0) stay on
            # a single memref for group/dependency tracking ----
            ps_at = psum.tile([128, 32], F32, tag="ps_at")
            ps_g0a = psum.tile([128, 96], F32, tag="ps_g0a")
            ps_g0o = psum.tile([128, 32], F32, tag="ps_g0o")
            ps_g1a = psum.tile([128, 96], F32, tag="ps_g1a")
            ps_g1o = psum.tile([128, 32], F32, tag="ps_g1o")
            # ctx accumulates into two separate banks (dt 0-3 / dt 4-7) so
            # the first normalization multiply depends only on the first 64
            # ctx matmuls and overlaps the second half (psum deps are
            # whole-tile conservative, so one tile would serialize it).
            ps_ctxA = psum.tile([128, 32], F32, tag="ps_ctxA")
            ps_ctxB = psum.tile([128, 32], F32, tag="ps_ctxB")
            ps_out = psum.tile([128, 64], F32, tag="ps_out")

            def bias_mm(ps_region, bk, col_lo, col_hi, stop, start=False):
                # adds the per-gate bias pattern: out[p, c] += bk[c//8, p]
                nc.tensor.matmul(
                    ps_region,
                    lhsT=bk[0:16, 0:128],
                    rhs=ind[0:16, col_lo:col_hi],
                    start=start,
                    stop=stop,
                )

            def open_gates(ps_a, ps_o, bk):
                # the bias matmuls OPEN each bank's accumulation group (they
                # zero the bank), so the group close waits only on the last
                # Wi matmul and the cell starts one instruction sooner.
                bias_mm(ps_a[:, 0:96], bk, 0, 96, stop=False, start=True)
                bias_mm(ps_o[:, 0:32], bk, 96, 128, stop=False, start=True)

            def wh_mm(ps_a, ps_o, w, hT, start):
                # Wh @ h into the two gate psum banks. IMPORTANT: start=True
                # marks the whole 2KB PSUM bank ("zero region"), so each bank
                # gets exactly ONE open accumulation group (a-gates bank and
                # o-gate bank) — never two concurrent groups per bank.
                for j in range(4):
                    rhs = hT[:, j * 8 : j * 8 + 8]
                    for M in range(16):
                        if M < 12:
                            tgt = ps_a[:, M * 8 : M * 8 + 8]
                        else:
                            tgt = ps_o[:, (M - 12) * 8 : (M - 12) * 8 + 8]
                        nc.tensor.matmul(
                            tgt,
                            lhsT=w[:, j * 4 * H + 128 * M : j * 4 * H + 128 * M + 128],
                            rhs=rhs,
                            start=(start and j == 0 and M in (0, 12)),
                            stop=False,
                        )

            def wi_mm(ps_a, ps_o, w, nk, rhs_of, bk, close_bias=False):
                # Wi @ x joins the open group per bank. With close_bias the
                # bias matmul closes the group (legacy); otherwise the group
                # was opened by the bias and the last Wi matmul closes it.
                dve = abl == "dvebias"

                def region(lo, hi, ps, coff):
                    for j in range(nk):
                        rhs = rhs_of(j)
                        for M in range(lo, hi):
                            nc.tensor.matmul(
                                ps[:, (M - coff) * 8 : (M - coff) * 8 + 8],
                                lhsT=w[
                                    :,
                                    j * 4 * H + 128 * M : j * 4 * H + 128 * M + 128,
                                ],
                                rhs=rhs,
                                start=False,
                                stop=(
                                    not close_bias
                                    and j == nk - 1
                                    and M == hi - 1
                                ),
                            )
                    if close_bias and not dve:
                        bias_mm(
                            ps[:, (lo - coff) * 8 : (hi - coff) * 8],
                            bk,
                            lo * 8,
                            hi * 8,
                            stop=True,
                        )

                region(0, 12, ps_a, 0)
                region(12, 16, ps_o, 12)

            def cell(ps_ga, ps_go, cT, hT, tag, bf_=None):
                if abl == "pefloor":
                    return
                if abl == "dvebias":
                    ga = work.tile([128, 96], F32, tag=f"ga{tag}")
                    nc.vector.tensor_add(ga, ps_ga, bf_[:, 0:96])
                    go = work.tile([128, 32], F32, tag=f"go{tag}")
                    nc.vector.tensor_add(go, ps_go, bf_[:, 96:128])
                    ps_ga, ps_go = ga, go
                if abl == "oldcell":
                    # baseline cell math: undoubled states, f32 intermediates
                    ta = work.tile([128, 96], F32, tag=f"ta{tag}")
                    nc.scalar.activation(
                        ta[:, 0:64], ps_ga[:, 0:64], TANH, scale=0.5
                    )
                    nc.scalar.activation(ta[:, 64:96], ps_ga[:, 64:96], TANH)
                    to = work.tile([128, 32], F32, tag=f"to{tag}")
                    nc.scalar.activation(to, ps_go, TANH, scale=0.5)
                    sif = work.tile([128, 64], F32, tag=f"sif{tag}")
                    nc.vector.tensor_scalar(sif, ta[:, 0:64], 0.5, 0.5, MULT, ADD)
                    so = work.tile([128, 32], F32, tag=f"so{tag}")
                    nc.vector.tensor_scalar(so, to, 0.5, 0.5, MULT, ADD)
                    v = work.tile([128, 32], F32, tag=f"v{tag}")
                    nc.vector.tensor_mul(v, sif[:, 0:32], ta[:, 64:96])
                    u = work.tile([128, 32], F32, tag=f"u{tag}")
                    nc.vector.tensor_mul(u, sif[:, 32:64], cT)
                    nc.vector.tensor_add(cT, u, v)
                    tc2 = work.tile([128, 32], F32, tag=f"tc2{tag}")
                    nc.scalar.activation(tc2, cT, TANH)
                    nc.vector.tensor_mul(hT, so, tc2)
                    return
                # LSTM cell from biased gate psums (i/f in ga[:,0:64], g in
                # ga[:,64:96], o in go). States are doubled: cT = 2c, hT = 2h
                # (weights consuming h are pre-halved on the host), which lets
                # sigmoid(x) = (tanh(x/2) + 1)/2 fuse into 3 DVE ops:
                #   v2 = (tanh(i/2)+1)*tanh(g)      = 2*sig(i)*tanh(g)
                #   u4 = (tanh(f/2)+1)*cT           = 4*sig(f)*c
                #   cT' = u4*0.5 + v2               = 2c'
                #   hT  = (tanh(o/2)+1)*tanh(cT'/2) = 2h'
                ta = work.tile([128, 96], F32, tag=f"ta{tag}")
                nc.scalar.activation(ta[:, 0:64], ps_ga[:, 0:64], TANH, scale=0.5)
                nc.scalar.activation(ta[:, 64:96], ps_ga[:, 64:96], TANH)
                to = work.tile([128, 32], F32, tag=f"to{tag}")
                nc.scalar.activation(to, ps_go, TANH, scale=0.5)
                v2 = work.tile([128, 32], F32, tag=f"v2{tag}")
                nc.vector.scalar_tensor_tensor(
                    v2, ta[:, 0:32], 1.0, ta[:, 64:96], ADD, MULT
                )
                u4 = work.tile([128, 32], F32, tag=f"u4{tag}")
                nc.vector.scalar_tensor_tensor(
                    u4, ta[:, 32:64], 1.0, cT, ADD, MULT
                )
                nc.vector.scalar_tensor_tensor(cT, u4, 0.5, v2, MULT, ADD)
                tc2 = work.tile([128, 32], F32, tag=f"tc2{tag}")
                nc.scalar.activation(tc2, cT, TANH, scale=0.5)
                if abl == "f32stt":
                    hf = work.tile([128, 32], F32, tag=f"hf{tag}")
                    nc.vector.scalar_tensor_tensor(hf, to, 1.0, tc2, ADD, MULT)
                    nc.vector.tensor_copy(hT, hf)
                else:
                    nc.vector.scalar_tensor_tensor(hT, to, 1.0, tc2, ADD, MULT)

            def emit_out(t_out):
                # out = W_out @ h1 + b_out: [p=d%128, g*8+b], then a
                # feature-major store y[p, t*64 + g*8 + b]; the host
                # untransposes to [b, t, d]. No PE transpose on the path.
                for kc in range(4):
                    for g in range(8):
                        o = kc * IN + 128 * g
                        nc.tensor.matmul(
                            ps_out[:, g * 8 : g * 8 + 8],
                            lhsT=wou[:, o : o + 128],
                            rhs=h1[:, kc * 8 : kc * 8 + 8],
                            start=(kc == 0 and g == 0),
                            stop=(kc == 3 and g == 7),
                        )
                if abl == "pefloor":
                    return
                outw = work.tile([128, 64], BF16, tag="outw")
                nc.vector.tensor_add(outw, ps_out, bo)
                if abl != "nodma":
                    nc.gpsimd.dma_start(
                        d_y[:, bass.ds(t_out * 64, 64)], outw
                    )

            def step(t_sv, wh0_here=False, emit_next_wh0=True, k_static=None,
                     lag_out=False):
                # scores are computed from [h0 ; h1] with W_out folded into
                # the attention weight (scores depend on out = Wou@h1 + bo
                # only linearly), so the output path leaves the critical path.
                first = (
                    (k_static == 0)
                    if k_static is not None
                    else (isinstance(t_sv, int) and t_sv == 0)
                )
                open_style = pipe == 2 and abl != "dvebias"
                if wh0_here:
                    wh_mm(ps_g0a, ps_g0o, wh0, h0, start=True)
                # ---- attention scores into ps_at[:, 0:16]; row sums 16:24 ----
                # h0 chunks first, then mask, then h1 chunks: the h1 part is
                # the only PE work gated on the previous cell1 chain.
                psc = ps_at[:, 0:16]
                for kc in range(8):  # 0-3: h0 chunks, 4-7: h1 (folded Wout)
                    for mt in range(2):
                        if kc < 4:
                            rhs = h0[:, kc * 8 : kc * 8 + 8]
                        else:
                            rhs = h1[:, (kc - 4) * 8 : (kc - 4) * 8 + 8]
                        o = kc * 2 * ENC // 2 + 128 * mt
                        nc.tensor.matmul(
                            psc[:, mt * 8 : mt * 8 + 8],
                            lhsT=wat[:, o : o + 128],
                            rhs=rhs,
                            start=(kc == 0 and mt == 0),
                            stop=(kc == 7 and mt == 1),
                        )
                    if kc == 3:
                        # mask as additive -300 on the scores (one rank-16
                        # matmul): masked entries then exp to ~0.
                        nc.tensor.matmul(
                            psc[:, 0:16],
                            lhsT=mneg[0:16, 0:128],
                            rhs=eye16[0:16, 0:16],
                            start=False,
                            stop=False,
                        )
                # ---- exp on ACT straight into the bf16 attention weights ----
                attnb = work.tile([128, 16], BF16, tag="attnb")
                if abl == "pefloor":
                    attnb = ones128[:, 0:16]  # skip exp: pure-PE diagnostic
                elif abl == "nodefer":
                    expf = work.tile([128, 16], F32, tag="expf")
                    expm = work.tile([128, 16], F32, tag="expm")
                    for mt in range(2):
                        nc.scalar.activation(
                            expf[:, mt * 8 : mt * 8 + 8],
                            psc[:, mt * 8 : mt * 8 + 8],
                            EXP,
                            bias=bat[:, mt : mt + 1],
                        )
                    nc.vector.tensor_mul(expm, expf, msk)
                else:
                    bcol = 2 if first else 0  # step 0: out==0, unfolded bias
                    for mt in range(2):
                        nc.scalar.activation(
                            attnb[:, mt * 8 : mt * 8 + 8],
                            psc[:, mt * 8 : mt * 8 + 8],
                            EXP,
                            bias=bat[:, bcol + mt : bcol + mt + 1],
                        )

                # ---- Wh1 part into gate psums (covers exp latency) ----
                if open_style:
                    open_gates(ps_g1a, ps_g1o, b1k)
                wh_mm(ps_g1a, ps_g1o, wh1, h1, start=not open_style)

                # ---- softmax row sums (partition-replicated) ----
                ps_s = ps_at[:, 16:24]
                if abl == "nodefer":
                    nc.tensor.matmul(ps_s, lhsT=ones128f, rhs=expm[:, 0:8],
                                     start=True, stop=False)
                    nc.tensor.matmul(ps_s, lhsT=ones128f, rhs=expm[:, 8:16],
                                     start=False, stop=True)
                    nc.vector.reciprocal(rec128, ps_s)
                    nc.vector.tensor_mul(attnb[:, 0:8], expm[:, 0:8], rec128)
                    nc.vector.tensor_mul(attnb[:, 8:16], expm[:, 8:16], rec128)
                else:
                    nc.tensor.matmul(ps_at[:, 16:32], lhsT=ones128,
                                     rhs=attnb[:, 0:16], start=True, stop=True)
                    if abl != "pefloor":
                        # DVE may read only one PSUM operand: stage one half
                        sumh = work.tile([128, 8], F32, tag="sumh")
                        nc.vector.tensor_copy(sumh, ps_at[:, 24:32])
                        sum8 = work.tile([128, 8], F32, tag="sum8")
                        nc.vector.tensor_add(sum8, ps_at[:, 16:24], sumh)
                        nc.vector.reciprocal(rec128, sum8)

                # ---- ctx[p=d%128, dt*8+b] = sum_e exp[b,e] enc[b,e,d] ----
                # dt-major, half-split across the two ctx banks.
                # kc outer / b inner: consecutive matmuls hit different psum
                # columns, so each column's kc=1 accumulate lands 8 matmuls
                # after its kc=0 write — back-to-back same-address PSUM
                # accumulation measures ~6ns/mm slower (mmbench `chain`).
                for dt_ in range(8):
                    ps_c = ps_ctxA if dt_ < 4 else ps_ctxB
                    co = dt_ * 8 if dt_ < 4 else (dt_ - 4) * 8
                    for kc in range(2):
                        for b in range(BL):
                            o = (b * 2 + kc) * IN + 128 * dt_
                            nc.tensor.matmul(
                                ps_c[:, co + b : co + b + 1],
                                lhsT=enc[:, o : o + 128],
                                rhs=attnb[:, kc * 8 + b : kc * 8 + b + 1],
                                start=(b == 0 and dt_ in (0, 4) and kc == 0),
                                stop=(b == BL - 1 and dt_ in (3, 7)
                                      and kc == 1),
                            )
                # normalization lands here (recip computed during ctx mms)
                ctxb = work.tile([128, 64], BF16, tag="ctxb")
                if abl == "pefloor":
                    ctxb = ones128[:, 0:64]
                elif abl == "nodefer":
                    nc.vector.tensor_copy(ctxb[:, 0:32], ps_ctxA)
                    nc.vector.tensor_copy(ctxb[:, 32:64], ps_ctxB)
                else:
                    # two halves so the first Wi0 K-chunks start sooner
                    for hl, ps_c in ((0, ps_ctxA), (1, ps_ctxB)):
                        nc.vector.tensor_mul(
                            _split_cols(ctxb[:, hl * 32 : hl * 32 + 32], 4, 8),
                            _split_cols(ps_c[:, 0:32], 4, 8),
                            _bcast_cols(rec128[:, 0:8], 4, 8),
                        )

                # ---- Wi0 @ ctx (joins the wh0 group opened last step) ----
                wi_mm(ps_g0a, ps_g0o, wi0, 8,
                      lambda j: ctxb[:, j * 8 : j * 8 + 8], b0k,
                      close_bias=not open_style)
                cell(ps_g0a, ps_g0o, c0, h0, "0", b0f)

                # ---- lagged output: W_out @ h1 still holds h1 of step t-1;
                # these 32 independent matmuls fill the cell0 -> Wi1 stall.
                if lag_out and not first:
                    emit_out(t_sv - 1)

                # ---- Wi1 @ h0_new ----
                wi_mm(ps_g1a, ps_g1o, wi1, 4,
                      lambda j: h0[:, j * 8 : j * 8 + 8], b1k,
                      close_bias=not open_style)

                # ---- Wh0 @ h0_new for the NEXT step (fills the cell gap) ----
                if emit_next_wh0:
                    if open_style:
                        open_gates(ps_g0a, ps_g0o, b0k)
                    wh_mm(ps_g0a, ps_g0o, wh0, h0, start=not open_style)

                cell(ps_g1a, ps_g1o, c1, h1, "1", b1f)

                if not lag_out:
                    emit_out(t_sv)

            if pipe == 2:
                # prologue: open the step-0 Wh0 group (h0 == 0)
                if abl != "dvebias":
                    open_gates(ps_g0a, ps_g0o, b0k)
                wh_mm(ps_g0a, ps_g0o, wh0, h0, start=abl == "dvebias")

            assert dec % unroll == 0
            niter = dec // unroll
            lag = pipe == 2 and niter == 1

            def body(iv):
                for k in range(unroll):
                    if pipe == 2:
                        step(iv * unroll + k, k_static=k, lag_out=lag)
                    elif pipe == 1:
                        step(iv * unroll + k, wh0_here=(k == 0),
                             emit_next_wh0=(k < unroll - 1))
                    else:
                        step(iv * unroll + k, wh0_here=True,
                             emit_next_wh0=False)

            hint_eng = (
                (mybir.EngineType.PE, mybir.EngineType.Activation,
                 mybir.EngineType.DVE)
                if hints
                else (mybir.EngineType.PE,)
            )
            if niter > 1 or loop_mult > 1:
                with tc.For_i(
                    0, niter * loop_mult, hint_engines=hint_eng
                ) as ivr:
                    iv = ivr % niter if loop_mult > 1 else ivr
                    body(iv)
            else:
                body(0)

            if pipe == 2:
                if lag:
                    # flush the last step's lagged output
                    emit_out(dec - 1)
                # epilogue: close + consume the dangling Wh0 group
                bias_mm(ps_g0a[:, 0:96], b0k, 0, 96, stop=True)
                bias_mm(ps_g0o[:, 0:32], b0k, 96, 128, stop=True)
                scrap = work.tile([128, 2], F32, tag="scrap")
                nc.vector.tensor_copy(scrap[:, 0:1], ps_g0a[:, 0:1])
                nc.vector.tensor_copy(scrap[:, 1:2], ps_g0o[:, 0:1])

    nc.compile()
    return nc


def prep_inputs(inputs, enc8=False, halve=True):
    """Host-side repack of the reference inputs into the kernel layouts."""
    gi = {k: np.asarray(v) for k, v in inputs.items()}
    bf = ml_dtypes.bfloat16
    edt = ml_dtypes.float8_e4m3 if enc8 else bf
    hs = 0.5 if halve else 1.0

    def kmajor(w, nk):
        # w: [M, K] -> [128, nk*M] with [p, kc*M + m] = w[m, 128*kc + p]
        M, K = w.shape
        assert K == nk * 128
        return np.ascontiguousarray(
            w.T.reshape(nk, 128, M).transpose(1, 0, 2).reshape(128, nk * M)
        )

    # The kernel keeps h-states doubled (hT = 2h); every weight that consumes
    # an h input is halved here so the matmul products stay exact.
    # W_out is folded into the attention weight: scores depend on the
    # previous step's out = Wou@h1 + bo only linearly, so
    #   scores = h0 @ Wa_h0.T + h1 @ (Wa_out @ Wou).T + (b_attn + Wa_out@bo)
    # (step 0 uses plain b_attn since out_0 = 0, not Wou@0 + bo).
    wa_out = gi["W_attn"][:, :IN].astype(np.float64)
    wa_h0 = gi["W_attn"][:, IN:].astype(np.float64)
    wfold = wa_out @ gi["W_out"].astype(np.float64)
    watn = np.concatenate([hs * wa_h0, hs * wfold], axis=1).astype(np.float32)
    batf = (gi["b_attn"] + wa_out @ gi["b_out"].astype(np.float64)).astype(
        np.float32
    )
    bat4 = np.concatenate(
        [
            np.ascontiguousarray(batf.reshape(2, 128).T),
            np.ascontiguousarray(gi["b_attn"].reshape(2, 128).T),
        ],
        axis=1,
    ).astype(np.float32)
    shared = {
        "wi0": kmajor(gi["W_ih0"], 8).astype(bf),
        "wh0": kmajor(hs * gi["W_hh0"], 4).astype(bf),
        "wi1": kmajor(hs * gi["W_ih1"], 4).astype(bf),
        "wh1": kmajor(hs * gi["W_hh1"], 4).astype(bf),
        "wat": kmajor(watn, 8).astype(bf),
        "wou": kmajor(hs * gi["W_out"], 4).astype(bf),
        "bat": bat4,
        "b0k": (gi["b_ih0"] + gi["b_hh0"]).reshape(16, 128).astype(bf),
        "b1k": (gi["b_ih1"] + gi["b_hh1"]).reshape(16, 128).astype(bf),
        "ind": np.repeat(np.eye(16, dtype=np.float32), 8, axis=1).astype(bf),
        "bo": np.repeat(gi["b_out"].reshape(8, 128).T, 8, axis=1).astype(
            np.float32
        ),
        "id128": np.eye(128, dtype=np.float32).astype(bf),
        "b0f": np.repeat(
            (gi["b_ih0"] + gi["b_hh0"]).reshape(16, 128).T, 8, axis=1
        ).astype(np.float32),
        "b1f": np.repeat(
            (gi["b_ih1"] + gi["b_hh1"]).reshape(16, 128).T, 8, axis=1
        ).astype(np.float32),
        "idf": np.eye(128, dtype=np.float32),
        "eye16": np.eye(16, dtype=np.float32).astype(bf),
    }
    in_maps = []
    for c in range(NCORES):
        e = gi["encoder2_hiddens"][c * BL : (c + 1) * BL]  # [8, 256, 1024]
        enc_t = np.ascontiguousarray(
            e.reshape(BL, 2, 128, IN).transpose(2, 0, 1, 3).reshape(128, BL * 2 * IN)
        ).astype(edt)
        m = gi["x2_mask"][c * BL : (c + 1) * BL]  # [8, 256] int32
        mf = (1 - m).astype(np.float32).T  # [256, 8]
        msk = np.ascontiguousarray(
            mf.reshape(2, 128, BL).transpose(1, 0, 2).reshape(128, 16)
        ).astype(np.float32)
        # additive score mask: mneg[mt*8+b, p] = -300 where e=mt*128+p masked
        mneg = (
            -300.0 * m.reshape(BL, 2, 128).transpose(1, 0, 2).reshape(16, 128)
        ).astype(bf)
        in_maps.append({**shared, "enc": enc_t, "msk": msk, "mneg": mneg})
    return in_maps


_cache = {}


def _get_nc(dec=DEC, unroll=2, loop_mult=1, enc8=False, pipe=2, abl="",
            hints=0):
    key = (dec, unroll, loop_mult, enc8, pipe, abl, hints)
    if key not in _cache:
        _cache[key] = build_nc(dec, unroll, loop_mult, enc8, pipe, abl, hints)
    return _cache[key]


class Runner:
    """Jit-compiles the Bass program once; repeat calls reuse the executable
    and the device-resident input shards (only fresh output buffers are
    shipped per call when donation is enabled)."""

    def __init__(self, nc, donate=True):
        import jax
        from concourse import bass2jax
        from jax.experimental.shard_map import shard_map
        from jax.sharding import Mesh, PartitionSpec

        bass2jax.install_neuronx_cc_hook()
        self.jax = jax
        self.nc = nc
        self.donate = donate
        pname = nc.partition_id_tensor.name if nc.partition_id_tensor else None
        in_names, out_names, out_avals, zero_outs = [], [], [], []
        self.in_shapes = {}
        for alloc in nc.m.functions[0].allocations:
            if not isinstance(alloc, mybir.MemoryLocationSet):
                continue
            name = alloc.memorylocations[0].name
            if alloc.kind == "ExternalInput":
                if name != pname:
                    in_names.append(name)
                    self.in_shapes[name] = (
                        tuple(alloc.tensor_shape),
                        mybir.dt.np(alloc.dtype),
                    )
            elif alloc.kind == "ExternalOutput":
                shape = tuple(alloc.tensor_shape)
                dtype = mybir.dt.np(alloc.dtype)
                out_names.append(name)
                out_avals.append(jax.core.ShapedArray(shape, dtype))
                zero_outs.append(np.zeros(shape, dtype))
        self.in_names, self.out_names = in_names, out_names
        self.out_avals, self.zero_outs = out_avals, zero_outs
        n_params, n_outs = len(in_names), len(out_names)
        all_names = in_names + out_names + ([pname] if pname else [])

        def _body(*args):
            operands = list(args)
            if pname is not None:
                operands.append(bass2jax.partition_id_tensor())
            outs = bass2jax._bass_exec_p.bind(
                *operands,
                out_avals=tuple(out_avals),
                in_names=tuple(all_names),
                out_names=tuple(out_names),
                lowering_input_output_aliases=(),
                sim_require_finite=True,
                sim_require_nnan=True,
                nc=nc,
            )
            return tuple(outs)

        devices = jax.devices()[:NCORES]
        assert len(devices) == NCORES
        self.mesh = Mesh(np.asarray(devices), ("core",))
        in_specs = (PartitionSpec("core"),) * (n_params + n_outs)
        out_specs = (PartitionSpec("core"),) * n_outs
        kw = (
            dict(donate_argnums=tuple(range(n_params, n_params + n_outs)))
            if donate
            else {}
        )
        self.fn = jax.jit(
            shard_map(
                _body, mesh=self.mesh, in_specs=in_specs, out_specs=out_specs,
                check_rep=False,
            ),
            keep_unused=True,
            **kw,
        )
        self._dev_in = None

    def _globalize(self, in_maps):
        jax = self.jax
        from jax.sharding import NamedSharding, PartitionSpec

        sh = NamedSharding(self.mesh, PartitionSpec("core"))
        arrs = []
        for name in self.in_names:
            if name in in_maps[0]:
                g = np.concatenate(
                    [np.asarray(m[name]) for m in in_maps], axis=0
                )
            else:  # auto-fill (e.g. the cache-bust tensor)
                shape, dt = self.in_shapes[name]
                g = np.zeros((NCORES * shape[0], *shape[1:]), dt)
            arrs.append(jax.device_put(g, sh))
        return arrs

    def set_inputs(self, in_maps):
        self._dev_in = self._globalize(in_maps)

    def _zeros_dev(self):
        from jax.sharding import NamedSharding, PartitionSpec

        sh = NamedSharding(self.mesh, PartitionSpec("core"))
        return [
            self.jax.device_put(
                np.zeros((NCORES * z.shape[0], *z.shape[1:]), z.dtype), sh
            )
            for z in self.zero_outs
        ]

    def __call__(self):
        outs = self.fn(*self._dev_in, *self._zeros_dev())
        return outs

    def gather(self, outs):
        res = []
        for i, name in enumerate(self.out_names):
            a = np.asarray(outs[i])
            res.append(a.reshape(NCORES, *self.out_avals[i].shape))
        return dict(zip(self.out_names, res))


_runner_cache = {}


def get_runner(dec=DEC, unroll=2, donate=True, loop_mult=1, enc8=False, pipe=2,
               abl="", hints=0):
    key = (dec, unroll, donate, loop_mult, enc8, pipe, abl, hints)
    if key not in _runner_cache:
        _runner_cache[key] = Runner(
            _get_nc(dec, unroll, loop_mult, enc8, pipe, abl, hints),
            donate=donate,
        )
    return _runner_cache[key]


# Unrolling amortizes hardware-loop boundary syncs: 2->4->8 bought ~9-10%
# per doubling, 8->16->32 a further ~1-2% each (verified by interleaved
# A/B on HW; math is bit-identical across unroll factors).
CFG = dict(unroll=128, enc8=False, pipe=2, hints=0)


def run_on_hw(inputs, dec=DEC, unroll=None, enc8=None, pipe=None, hints=None):
    unroll = CFG["unroll"] if unroll is None else unroll
    enc8 = CFG["enc8"] if enc8 is None else enc8
    pipe = CFG["pipe"] if pipe is None else pipe
    hints = CFG["hints"] if hints is None else hints
    r = get_runner(dec, unroll, enc8=enc8, pipe=pipe, hints=hints)
    r.set_inputs(prep_inputs(inputs, enc8=enc8))
    outs = r()
    y = r.gather(outs)["y"]  # [NCORES, 128, dec*64] bf16
    # y[c, p, t*64 + g*8 + b] -> out[c*BL + b, t, 128*g + p]
    y = y.reshape(NCORES, 128, dec, 8, BL).transpose(0, 4, 2, 3, 1)
    return np.ascontiguousarray(y).reshape(B, dec, IN).astype(np.float32)


def kernel(**inputs):
    return run_on_hw(inputs)

